# revision 7
# baseline (speedup 1.0000x reference)
"""Causal self-attention (B=4, T=2048, D=1024, H=16, head_dim=64) on 8 TRN2
NeuronCores.

Sharding: core c handles batch b = c//2 and head-half hh = c%2 (8 heads).
Each core computes its batch's QKV projection restricted to its heads, RoPE,
causal attention for its 8 heads, and a partial output projection against its
512 rows of w_out. The host sums the two partials per batch.

On-device layout (per core):
  - everything runs transposed: q^T/k^T [head_dim, T] so that the S^T = K^T Q
    matmuls need no transposes, and P@V is computed as V^T P^T with V as the
    stationary operand.  A row of ones appended to V yields the softmax
    denominators in the same PSUM accumulation (row 64), so softmax needs no
    partition-axis reduction.
  - softmax skips max-subtraction: causal scores for this problem are O(10),
    exp(s/8) is safely in fp32 range.
  - matmuls run as float32r (single-pass reduced-precision fp32, ~1e-4 rel).
  - RoPE's rotate-half is a partition permutation, done with one extra matmul
    against a constant +-1 permutation matrix instead of partition-shifted
    vector ops.
"""
import os
import sys

sys.path.insert(0, "/opt/trn_rl_repo")

import numpy as np

import concourse.bass as bass
import concourse.mybir as mybir
import concourse.tile as tile
from concourse import bacc
from concourse.bass_utils import run_bass_kernel_spmd

F32 = mybir.dt.float32
F32R = mybir.dt.float32r
EXP = mybir.ActivationFunctionType.Exp
ADD = mybir.AluOpType.add

B, T, DIM, HEADS, HD = 4, 2048, 1024, 16, 64
THETA = 10000.0
NCORES = 8
NEG = -1.0e9


def _consts():
    """Host-side constant tensors shared by all cores."""
    freqs = 1.0 / THETA ** (np.arange(0, HD, 2, dtype=np.float32) / HD)  # [32]
    t = np.arange(T, dtype=np.float32)
    ang = t[None, :] * freqs[np.arange(128) % 32, None]  # [128, T]
    cosT = np.cos(ang).astype(np.float32)
    sinT = np.sin(ang).astype(np.float32)

    P = np.zeros((128, 128), dtype=np.float32)
    for i in range(128):
        base, il = (i // 64) * 64, i % 64
        if il < 32:
            P[i, base + il + 32] = -1.0
        else:
            P[i, base + il - 32] = 1.0
    PT = P.T.copy()

    k = np.arange(128)[:, None]
    q = np.arange(128)[None, :]
    tri = np.where(k <= q, 0.0, NEG).astype(np.float32)  # [128,128]
    trij3 = np.full((128, 256), NEG, dtype=np.float32)
    trij3[:, 128:] = tri
    return cosT, sinT, PT, tri, trij3


def _build(repeat=1):
    nc = bacc.Bacc("TRN2", target_bir_lowering=False, debug=False)

    xT = nc.dram_tensor("xT", [DIM, T], F32R, kind="ExternalInput")
    wqk = nc.dram_tensor("wqk", [DIM, 1024], F32R, kind="ExternalInput")
    wv = nc.dram_tensor("wv", [DIM, 512], F32R, kind="ExternalInput")
    wo = nc.dram_tensor("wo", [512, DIM], F32R, kind="ExternalInput")
    cosT_d = nc.dram_tensor("cosT", [128, T], F32, kind="ExternalInput")
    sinT_d = nc.dram_tensor("sinT", [128, T], F32, kind="ExternalInput")
    PT_d = nc.dram_tensor("PT", [128, 128], F32R, kind="ExternalInput")
    tri_d = nc.dram_tensor("tri", [128, 128], F32, kind="ExternalInput")
    trij3_d = nc.dram_tensor("trij3", [128, 256], F32, kind="ExternalInput")
    ones_d = nc.dram_tensor("ones", [128, 128], F32R, kind="ExternalInput")
    outp = nc.dram_tensor("outp", [T, DIM], F32, kind="ExternalOutput")

    with tile.TileContext(nc) as tc:
      for _rep in range(repeat):
        with tc.tile_pool(name="glob", bufs=1) as glob:
            # whole-kernel tensors
            q_rope = glob.tile([128, 4, T], F32R)   # [part, pair, t]
            k_rope = glob.tile([128, 4, T], F32R)
            v_aug = glob.tile([128, 16, 8, 65], F32R)  # [t%128, tchunk, head, d+1]
            tri_sb = glob.tile([128, 128], F32)
            trij3_sb = glob.tile([128, 256], F32)
            nc.sync.dma_start(out=tri_sb, in_=tri_d[:])
            nc.sync.dma_start(out=trij3_sb, in_=trij3_d[:])
            nc.sync.dma_start(
                out=v_aug[:, :, :, 64:65],
                in_=ones_d.rearrange("p (a b o) -> p a b o", a=16, o=1),
            )

            # ---------------- Phase 1: projections + rope ----------------
            with (
                tc.tile_pool(name="p1", bufs=1) as p1,
                tc.tile_pool(name="p1x", bufs=2) as p1x,
                tc.tile_pool(name="p1t", bufs=2) as p1t,
                tc.tile_pool(name="p1ps", bufs=3, space="PSUM") as p1ps,
                tc.tile_pool(name="p1rot", bufs=2, space="PSUM") as p1rot,
            ):
                wqk_sb = p1.tile([128, 8, 1024], F32R)
                wv_sb = p1.tile([128, 8, 512], F32R)
                cos_sb = p1.tile([128, T], F32)
                sin_sb = p1.tile([128, T], F32)
                PT_sb = p1.tile([128, 128], F32R)
                nc.sync.dma_start(out=wqk_sb, in_=wqk.rearrange("(c p) m -> p c m", p=128))
                nc.sync.dma_start(out=wv_sb, in_=wv.rearrange("(c p) m -> p c m", p=128))
                nc.sync.dma_start(out=cos_sb, in_=cosT_d[:])
                nc.sync.dma_start(out=sin_sb, in_=sinT_d[:])
                nc.sync.dma_start(out=PT_sb, in_=PT_d[:])

                xr = xT.rearrange("(c p) t -> p c t", p=128)
                for n in range(4):
                    ncol = slice(n * 512, (n + 1) * 512)
                    x_t = p1x.tile([128, 8, 512], F32R)
                    nc.sync.dma_start(out=x_t, in_=xr[:, :, ncol])
                    # q (m 0..3) and k (m 4..7) projections, transposed
                    for m in range(8):
                        ps = p1ps.tile([128, 512], F32, tag="proj")
                        for k in range(8):
                            nc.tensor.matmul(
                                ps[:], wqk_sb[:, k, m * 128:(m + 1) * 128],
                                x_t[:, k, :], start=(k == 0), stop=(k == 7),
                            )
                        raw = p1t.tile([128, 512], F32R, tag="raw")
                        nc.vector.tensor_copy(raw[:], ps[:])
                        rotp = p1rot.tile([128, 512], F32)
                        nc.tensor.matmul(rotp[:], PT_sb[:], raw[:], start=True, stop=True)
                        t1 = p1t.tile([128, 512], F32, tag="t1")
                        t2 = p1t.tile([128, 512], F32, tag="t2")
                        nc.vector.tensor_mul(t1[:], raw[:], cos_sb[:, ncol])
                        nc.vector.tensor_mul(t2[:], rotp[:], sin_sb[:, ncol])
                        dest = q_rope if m < 4 else k_rope
                        nc.vector.tensor_add(dest[:, m % 4, ncol], t1[:], t2[:])
                    # v projection for this T-block ([T, vdim] orientation)
                    for ts in range(4):
                        psv = p1ps.tile([128, 512], F32, tag="proj")
                        for k in range(8):
                            nc.tensor.matmul(
                                psv[:], x_t[:, k, ts * 128:(ts + 1) * 128],
                                wv_sb[:, k, :], start=(k == 0), stop=(k == 7),
                            )
                        nc.vector.tensor_copy(
                            v_aug[:, n * 4 + ts, :, 0:64],
                            psv.rearrange("p (h d) -> p h d", h=8),
                        )

            # ---------------- Phase 2: attention ----------------
            with (
                tc.tile_pool(name="p2", bufs=1) as p2,
                tc.tile_pool(name="p2pt", bufs=3) as p2pt,
                tc.tile_pool(name="p2n", bufs=4) as p2n,
                tc.tile_pool(name="p2st", bufs=2, space="PSUM") as p2st,
                tc.tile_pool(name="p2o", bufs=4, space="PSUM") as ps_out,
                tc.tile_pool(name="p2dram", bufs=8, space="DRAM") as p2dram,
            ):
                att = p2.tile([128, 4, T], F32R)  # att_norm^T [attdim, t]
                for p in range(4):
                    for qb in range(4):
                        qcol = lambda c0: slice(qb * 512 + c0, (qb + 1) * 512)
                        nkc = 4 * qb + 4
                        oA = ps_out.tile([65, 512], F32, tag="o")
                        oB = ps_out.tile([65, 512], F32, tag="o")
                        for kc in range(nkc):
                            j = kc - 4 * qb
                            c0 = 0 if j < 0 else (256 if j == 3 else 128 * j)
                            kcol = slice(kc * 128, (kc + 1) * 128)
                            st = p2st.tile([128, 1024], F32)
                            nc.tensor.matmul(
                                st[:, c0:512], k_rope[0:64, p, kcol],
                                q_rope[0:64, p, qcol(c0)], start=True, stop=True,
                            )
                            nc.tensor.matmul(
                                st[:, 512 + c0:1024], k_rope[64:128, p, kcol],
                                q_rope[64:128, p, qcol(c0)], start=True, stop=True,
                            )
                            if j >= 0:
                                if j < 3:
                                    ms = slice(128 * j, 128 * j + 128)
                                    mk = tri_sb
                                else:
                                    ms = slice(256, 512)
                                    mk = trij3_sb
                                nc.vector.tensor_tensor(st[:, ms], st[:, ms], mk[:], ADD)
                                ms2 = slice(512 + ms.start, 512 + ms.stop)
                                nc.vector.tensor_tensor(st[:, ms2], st[:, ms2], mk[:], ADD)
                            pt = p2pt.tile([128, 1024], F32R)
                            nc.scalar.activation(
                                pt[:, c0:1024], st[:, c0:1024], EXP,
                                bias=0.0, scale=0.125,
                            )
                            nc.tensor.matmul(
                                oA[:, c0:512], v_aug[:, kc, 2 * p, :],
                                pt[:, c0:512], start=(kc == 0),
                                stop=(kc == nkc - 1), skip_group_check=True,
                            )
                            nc.tensor.matmul(
                                oB[:, c0:512], v_aug[:, kc, 2 * p + 1, :],
                                pt[:, 512 + c0:1024], start=(kc == 0),
                                stop=(kc == nkc - 1), skip_group_check=True,
                            )
                        # normalize: att[h rows, qb cols] = o[0:64] * (1/o[64])
                        for hi, o in ((2 * p, oA), (2 * p + 1, oB)):
                            rsum = p2n.tile([1, 512], F32, tag="rsum")
                            nc.vector.reciprocal(rsum[:], o[64:65, :])
                            scr = p2dram.tile([1, 512], F32)
                            nc.sync.dma_start(out=scr[:], in_=rsum[:])
                            rbc = p2n.tile([64, 512], F32, tag="rbc")
                            sap = scr[:]
                            nc.sync.dma_start(
                                out=rbc[:],
                                in_=bass.AP(tensor=sap.tensor, offset=sap.offset,
                                            ap=[[0, 64], [1, 512]]),
                            )
                            nc.vector.tensor_mul(
                                att[(hi % 2) * 64:(hi % 2) * 64 + 64, hi // 2,
                                    qb * 512:(qb + 1) * 512],
                                o[0:64, :], rbc[:],
                            )

                # ---------------- Phase 3: output projection ----------------
                with tc.tile_pool(name="p3", bufs=1) as p3:
                    wo_sb = p3.tile([128, 4, 1024], F32R)
                    nc.sync.dma_start(out=wo_sb, in_=wo.rearrange("(c p) m -> p c m", p=128))
                    for tcb in range(16):
                        for od in range(2):
                            po = ps_out.tile([128, 512], F32, tag="o")
                            for ac in range(4):
                                nc.tensor.matmul(
                                    po[:], att[:, ac, tcb * 128:(tcb + 1) * 128],
                                    wo_sb[:, ac, od * 512:(od + 1) * 512],
                                    start=(ac == 0), stop=(ac == 3),
                                )
                            ob = p2n.tile([128, 512], F32, tag="ob")
                            nc.vector.tensor_copy(ob[:], po[:])
                            nc.sync.dma_start(
                                out=outp[tcb * 128:(tcb + 1) * 128,
                                         od * 512:(od + 1) * 512],
                                in_=ob[:],
                            )
    nc.compile()
    return nc


_NC = {}


def _get_nc(repeat=1):
    if repeat not in _NC:
        _NC[repeat] = _build(repeat)
    return _NC[repeat]


def kernel(x, w_qkv, w_out):
    x = np.ascontiguousarray(x, dtype=np.float32)
    w_qkv = np.ascontiguousarray(w_qkv, dtype=np.float32)
    w_out = np.ascontiguousarray(w_out, dtype=np.float32)

    cosT, sinT, PT, tri, trij3 = _consts()
    in_maps = []
    for c in range(NCORES):
        b, hh = c // 2, c % 2
        wqk = np.ascontiguousarray(np.concatenate(
            [w_qkv[:, 512 * hh:512 * hh + 512],
             w_qkv[:, 1024 + 512 * hh:1024 + 512 * hh + 512]], axis=1))
        wv = np.ascontiguousarray(w_qkv[:, 2048 + 512 * hh:2048 + 512 * hh + 512])
        wo = np.ascontiguousarray(w_out[512 * hh:512 * hh + 512, :])
        xTb = np.ascontiguousarray(x[b].T)
        in_maps.append(dict(xT=xTb, wqk=wqk, wv=wv, wo=wo, cosT=cosT,
                            sinT=sinT, PT=PT, tri=tri, trij3=trij3,
                            ones=np.ones((128, 128), dtype=np.float32)))

    nc = _get_nc(int(os.environ.get("KREPEAT", "1")))
    r = run_bass_kernel_spmd(nc, in_maps, core_ids=list(range(NCORES)),
                             trace=bool(int(os.environ.get("KTRACE", "0"))))
    out = np.empty((B, T, DIM), dtype=np.float32)
    for b in range(B):
        out[b] = r.results[2 * b]["outp"] + r.results[2 * b + 1]["outp"]
    kernel.last_results = r
    return out


# revision 10
# speedup vs baseline: 1.8108x; 1.8108x over previous
"""Causal self-attention (B=4, T=2048, D=1024, H=16, head_dim=64) on 8 TRN2
NeuronCores.

Sharding: core c handles batch b = c//2 and head-half hh = c%2 (8 heads).
Each core computes its batch's QKV projection restricted to its heads, RoPE,
causal attention for its 8 heads, and a partial output projection against its
512 rows of w_out. The host sums the two partials per batch.

On-device layout (per core):
  - everything runs transposed: q^T/k^T [head_dim, T] so that the S^T = K^T Q
    matmuls need no transposes, and P@V is computed as V^T P^T with V as the
    stationary operand.  A row of ones appended to V yields the softmax
    denominators in the same PSUM accumulation (row 64), so softmax needs no
    partition-axis reduction.
  - softmax skips max-subtraction: causal scores for this problem are O(10),
    exp(s/8) is safely in fp32 range.
  - matmuls run as float32r (single-pass reduced-precision fp32, ~1e-4 rel).
  - RoPE's rotate-half is a partition permutation, done with one extra matmul
    against a constant +-1 permutation matrix instead of partition-shifted
    vector ops.
"""
import os
import sys

sys.path.insert(0, "/opt/trn_rl_repo")

import numpy as np

import concourse.bass as bass
import concourse.mybir as mybir
import concourse.tile as tile
from concourse import bacc
from concourse.bass_utils import run_bass_kernel_spmd

F32 = mybir.dt.float32
F32R = mybir.dt.float32r
EXP = mybir.ActivationFunctionType.Exp
ADD = mybir.AluOpType.add

B, T, DIM, HEADS, HD = 4, 2048, 1024, 16, 64
THETA = 10000.0
NCORES = 8
NEG = -1.0e9


def _consts():
    """Host-side constant tensors shared by all cores."""
    freqs = 1.0 / THETA ** (np.arange(0, HD, 2, dtype=np.float32) / HD)  # [32]
    t = np.arange(T, dtype=np.float32)
    ang = t[None, :] * freqs[np.arange(128) % 32, None]  # [128, T]
    cosT = np.cos(ang).astype(np.float32)
    sinT = np.sin(ang).astype(np.float32)

    P = np.zeros((128, 128), dtype=np.float32)
    for i in range(128):
        base, il = (i // 64) * 64, i % 64
        if il < 32:
            P[i, base + il + 32] = -1.0
        else:
            P[i, base + il - 32] = 1.0
    PT = P.T.copy()

    k = np.arange(128)[:, None]
    q = np.arange(128)[None, :]
    tri = np.where(k <= q, 0.0, NEG).astype(np.float32)  # [128,128]
    trij3 = np.full((128, 256), NEG, dtype=np.float32)
    trij3[:, 128:] = tri
    return cosT, sinT, PT, tri, trij3


def _build(repeat=1):
    nc = bacc.Bacc("TRN2", target_bir_lowering=False, debug=False)

    xT = nc.dram_tensor("xT", [DIM, T], F32R, kind="ExternalInput")
    wqk = nc.dram_tensor("wqk", [DIM, 1024], F32R, kind="ExternalInput")
    wv = nc.dram_tensor("wv", [DIM, 512], F32R, kind="ExternalInput")
    wo = nc.dram_tensor("wo", [512, DIM], F32R, kind="ExternalInput")
    cosT_d = nc.dram_tensor("cosT", [128, T], F32, kind="ExternalInput")
    sinT_d = nc.dram_tensor("sinT", [128, T], F32, kind="ExternalInput")
    PT_d = nc.dram_tensor("PT", [128, 128], F32R, kind="ExternalInput")
    tri_d = nc.dram_tensor("tri", [128, 128], F32, kind="ExternalInput")
    trij3_d = nc.dram_tensor("trij3", [128, 256], F32, kind="ExternalInput")
    ones_d = nc.dram_tensor("ones", [128, 128], F32R, kind="ExternalInput")
    outp = nc.dram_tensor("outp", [T, DIM], F32, kind="ExternalOutput")

    with tile.TileContext(nc) as tc:
      for _rep in range(repeat):
        with tc.tile_pool(name="glob", bufs=1) as glob:
            # whole-kernel tensors
            q_rope = glob.tile([128, 4, T], F32R)   # [part, pair, t]
            k_rope = glob.tile([128, 4, T], F32R)
            v_aug = glob.tile([128, 16, 8, 65], F32R)  # [t%128, tchunk, head, d+1]
            tri_sb = glob.tile([128, 128], F32)
            trij3_sb = glob.tile([128, 256], F32)
            nc.sync.dma_start(out=tri_sb, in_=tri_d[:])
            nc.sync.dma_start(out=trij3_sb, in_=trij3_d[:])
            nc.sync.dma_start(
                out=v_aug[:, :, :, 64:65],
                in_=ones_d.rearrange("p (a b o) -> p a b o", a=16, o=1),
            )

            # ---------------- Phase 1: projections + rope ----------------
            with (
                tc.tile_pool(name="p1", bufs=1) as p1,
                tc.tile_pool(name="p1x", bufs=2) as p1x,
                tc.tile_pool(name="p1t", bufs=2) as p1t,
                tc.tile_pool(name="p1ps", bufs=3, space="PSUM") as p1ps,
                tc.tile_pool(name="p1rot", bufs=2, space="PSUM") as p1rot,
            ):
                wqk_sb = p1.tile([128, 8, 1024], F32R)
                wv_sb = p1.tile([128, 8, 512], F32R)
                cos_sb = p1.tile([128, T], F32)
                sin_sb = p1.tile([128, T], F32)
                PT_sb = p1.tile([128, 128], F32R)
                nc.sync.dma_start(out=wqk_sb, in_=wqk.rearrange("(c p) m -> p c m", p=128))
                nc.sync.dma_start(out=wv_sb, in_=wv.rearrange("(c p) m -> p c m", p=128))
                nc.sync.dma_start(out=cos_sb, in_=cosT_d[:])
                nc.sync.dma_start(out=sin_sb, in_=sinT_d[:])
                nc.sync.dma_start(out=PT_sb, in_=PT_d[:])

                xr = xT.rearrange("(c p) t -> p c t", p=128)
                for n in range(4):
                    ncol = slice(n * 512, (n + 1) * 512)
                    x_t = p1x.tile([128, 8, 512], F32R)
                    nc.sync.dma_start(out=x_t, in_=xr[:, :, ncol])
                    # q (m 0..3) and k (m 4..7) projections, transposed
                    for m in range(8):
                        ps = p1ps.tile([128, 512], F32, tag="proj")
                        for k in range(8):
                            nc.tensor.matmul(
                                ps[:], wqk_sb[:, k, m * 128:(m + 1) * 128],
                                x_t[:, k, :], start=(k == 0), stop=(k == 7),
                            )
                        raw = p1t.tile([128, 512], F32R, tag="raw")
                        nc.vector.tensor_copy(raw[:], ps[:])
                        rotp = p1rot.tile([128, 512], F32)
                        nc.tensor.matmul(rotp[:], PT_sb[:], raw[:], start=True, stop=True)
                        t1 = p1t.tile([128, 512], F32, tag="t1")
                        t2 = p1t.tile([128, 512], F32, tag="t2")
                        nc.vector.tensor_mul(t1[:], raw[:], cos_sb[:, ncol])
                        nc.vector.tensor_mul(t2[:], rotp[:], sin_sb[:, ncol])
                        dest = q_rope if m < 4 else k_rope
                        nc.vector.tensor_add(dest[:, m % 4, ncol], t1[:], t2[:])
                    # v projection for this T-block ([T, vdim] orientation)
                    for ts in range(4):
                        psv = p1ps.tile([128, 512], F32, tag="proj")
                        for k in range(8):
                            nc.tensor.matmul(
                                psv[:], x_t[:, k, ts * 128:(ts + 1) * 128],
                                wv_sb[:, k, :], start=(k == 0), stop=(k == 7),
                            )
                        nc.vector.tensor_copy(
                            v_aug[:, n * 4 + ts, :, 0:64],
                            psv.rearrange("p (h d) -> p h d", h=8),
                        )

            # ---------------- Phase 2: attention ----------------
            with (
                tc.tile_pool(name="p2", bufs=1) as p2,
                tc.tile_pool(name="p2pt", bufs=3) as p2pt,
                tc.tile_pool(name="p2n", bufs=4) as p2n,
                tc.tile_pool(name="p2st", bufs=2, space="PSUM") as p2st,
                tc.tile_pool(name="p2o", bufs=4, space="PSUM") as ps_out,
                tc.tile_pool(name="p2dram", bufs=8, space="DRAM") as p2dram,
            ):
                att = p2.tile([128, 4, T], F32R)  # att_norm^T [attdim, t]
                for p in range(4):
                    for qb in range(4):
                        qcol = lambda c0: slice(qb * 512 + c0, (qb + 1) * 512)
                        nkc = 4 * qb + 4
                        oA = ps_out.tile([65, 512], F32, tag="o")
                        oB = ps_out.tile([65, 512], F32, tag="o")
                        for kc in range(nkc):
                            j = kc - 4 * qb
                            c0 = 0 if j < 0 else (256 if j == 3 else 128 * j)
                            kcol = slice(kc * 128, (kc + 1) * 128)
                            st = p2st.tile([128, 1024], F32)
                            nc.tensor.matmul(
                                st[:, c0:512], k_rope[0:64, p, kcol],
                                q_rope[0:64, p, qcol(c0)], start=True, stop=True,
                            )
                            nc.tensor.matmul(
                                st[:, 512 + c0:1024], k_rope[64:128, p, kcol],
                                q_rope[64:128, p, qcol(c0)], start=True, stop=True,
                            )
                            if j >= 0:
                                if j < 3:
                                    ms = slice(128 * j, 128 * j + 128)
                                    mk = tri_sb
                                else:
                                    ms = slice(256, 512)
                                    mk = trij3_sb
                                nc.vector.tensor_tensor(st[:, ms], st[:, ms], mk[:], ADD)
                                ms2 = slice(512 + ms.start, 512 + ms.stop)
                                nc.vector.tensor_tensor(st[:, ms2], st[:, ms2], mk[:], ADD)
                            pt = p2pt.tile([128, 1024], F32R)
                            nc.scalar.activation(
                                pt[:, c0:1024], st[:, c0:1024], EXP,
                                bias=0.0, scale=0.125,
                            )
                            nc.tensor.matmul(
                                oA[:, c0:512], v_aug[:, kc, 2 * p, :],
                                pt[:, c0:512], start=(kc == 0),
                                stop=(kc == nkc - 1), skip_group_check=True,
                            )
                            nc.tensor.matmul(
                                oB[:, c0:512], v_aug[:, kc, 2 * p + 1, :],
                                pt[:, 512 + c0:1024], start=(kc == 0),
                                stop=(kc == nkc - 1), skip_group_check=True,
                            )
                        # normalize: att[h rows, qb cols] = o[0:64] * (1/o[64])
                        for hi, o in ((2 * p, oA), (2 * p + 1, oB)):
                            rsum = p2n.tile([1, 512], F32, tag="rsum")
                            nc.vector.reciprocal(rsum[:], o[64:65, :])
                            scr = p2dram.tile([1, 512], F32)
                            nc.sync.dma_start(out=scr[:], in_=rsum[:])
                            rbc = p2n.tile([64, 512], F32, tag="rbc")
                            sap = scr[:]
                            nc.sync.dma_start(
                                out=rbc[:],
                                in_=bass.AP(tensor=sap.tensor, offset=sap.offset,
                                            ap=[[0, 64], [1, 512]]),
                            )
                            nc.vector.tensor_mul(
                                att[(hi % 2) * 64:(hi % 2) * 64 + 64, hi // 2,
                                    qb * 512:(qb + 1) * 512],
                                o[0:64, :], rbc[:],
                            )

                # ---------------- Phase 3: output projection ----------------
                with tc.tile_pool(name="p3", bufs=1) as p3:
                    wo_sb = p3.tile([128, 4, 1024], F32R)
                    nc.sync.dma_start(out=wo_sb, in_=wo.rearrange("(c p) m -> p c m", p=128))
                    for tcb in range(16):
                        for od in range(2):
                            po = ps_out.tile([128, 512], F32, tag="o")
                            for ac in range(4):
                                nc.tensor.matmul(
                                    po[:], att[:, ac, tcb * 128:(tcb + 1) * 128],
                                    wo_sb[:, ac, od * 512:(od + 1) * 512],
                                    start=(ac == 0), stop=(ac == 3),
                                )
                            ob = p2n.tile([128, 512], F32, tag="ob")
                            nc.vector.tensor_copy(ob[:], po[:])
                            nc.sync.dma_start(
                                out=outp[tcb * 128:(tcb + 1) * 128,
                                         od * 512:(od + 1) * 512],
                                in_=ob[:],
                            )
    nc.compile()
    return nc


_NC = {}


def _get_nc(repeat=1):
    if repeat not in _NC:
        _NC[repeat] = _build(repeat)
    return _NC[repeat]


def _in_maps(x, w_qkv, w_out):
    cosT, sinT, PT, tri, trij3 = _consts()
    maps = []
    for c in range(NCORES):
        b, hh = c // 2, c % 2
        wqkm = np.ascontiguousarray(np.concatenate(
            [w_qkv[:, 512 * hh:512 * hh + 512],
             w_qkv[:, 1024 + 512 * hh:1024 + 512 * hh + 512]], axis=1))
        wvm = np.ascontiguousarray(w_qkv[:, 2048 + 512 * hh:2048 + 512 * hh + 512])
        wom = np.ascontiguousarray(w_out[512 * hh:512 * hh + 512, :])
        xTb = np.ascontiguousarray(x[b].T)
        maps.append(dict(xT=xTb, wqk=wqkm, wv=wvm, wo=wom, cosT=cosT,
                         sinT=sinT, PT=PT, tri=tri, trij3=trij3,
                         ones=np.ones((128, 128), dtype=np.float32)))
    return maps


def kernel(x, w_qkv, w_out):
    x = np.ascontiguousarray(x, dtype=np.float32)
    w_qkv = np.ascontiguousarray(w_qkv, dtype=np.float32)
    w_out = np.ascontiguousarray(w_out, dtype=np.float32)

    nc = _get_nc(int(os.environ.get("KREPEAT", "1")))
    r = run_bass_kernel_spmd(nc, _in_maps(x, w_qkv, w_out),
                             core_ids=list(range(NCORES)))
    out = np.empty((B, T, DIM), dtype=np.float32)
    for b in range(B):
        out[b] = r.results[2 * b]["outp"] + r.results[2 * b + 1]["outp"]
    kernel.last_results = r
    return out
